# revision 1
# baseline (speedup 1.0000x reference)
"""GATv2 (nn_GATv2_59184649339075) Bass kernel for TRN2, 8-core SPMD.

Self-contained: kernel(**inputs) takes the full unsharded inputs
(x[50000,64], W[64,64], b[64], a[64], edge_index[2,800000] int32) and
returns the full [50000,64] float32 output.

Design (v3, degree-bucketed dst-slot layout, no one-hots, no dst gather):
  Host: nodes are grouped into 400 dst-tiles of 128 by (per-node L/H edge
    counts); each core owns 50 tiles (tile rank r -> core r%8). Each dst
    node's edges occupy fixed columns of its tile: partition = dst node,
    column = edge slot, padded to per-position caps shared by all cores.
    The f16 node table holds 256B rows [a~*Wh | Wh] (a~ = |a| with the
    attention vector's sign folded into a feature reordering: F+ first).
    Table row r = p*400 + t; int16 gather indices address two overlapping
    windows (L: rows<32768, H: rows>=18432); per-tile partition placement
    puts high-src-degree nodes in the overlap so per-edge window choice
    can balance per-node L/H counts (pad factor ~1.08).
  Device per core: wh_stage computes the f16 table on PE (one [65,128]
    matmul per tile, bias folded) and writes it to DRAM; a second small
    pass computes the core's own 50 dst-tiles' scaled rows into SBUF.
    Edge phase per column-group: one dma_gather per window fetches
    [a~Wh|Wh] rows per edge; u = scaled_src + scaled_dst_tile (DVE f16
    2x, dst side broadcast by construction); Prelu on ACT; score
    e = sum(F+) - sum(F-) via pairwise f16 add-trees (2x); exp on ACT;
    v = raw_src * (masked exp) in-place (2x pair trick); per-tile
    aggregation and denominator via strided reduces; sigmoid(num/den)
    on ACT. Output layout [p, tile, f] f16, unpermuted on host.
"""
import sys

sys.path.insert(0, "/opt/trn_rl_repo")
from contextlib import ExitStack
from dataclasses import dataclass, field

import numpy as np

import concourse.bass as bass
import concourse.tile as tile
from concourse import bacc, mybir

F32 = mybir.dt.float32
F16 = mybir.dt.float16
I16 = mybir.dt.int16
AF = mybir.ActivationFunctionType

N_CORES = 8
P = 128
D = 64
NSLOPE = 0.2
N_TILES = 400
T_CORE = 50
NP_ = N_TILES * P          # 51200 padded nodes
LWIN = 32768               # L window rows [0, 32768)
HBASE = NP_ - 32768        # H window rows [18432, 51200)
GC = 112                   # max columns per edge group
SINGLE_PACKET = False      # SWDGE gather packet mode
GSPLIT = 2                 # gather pieces per window


@dataclass(frozen=True)
class Cfg:
    capL: tuple               # per-pos L caps (len 50)
    capH: tuple
    kpos: int                 # features with a >= 0 (F+ block size)
    groups: tuple             # tuple of (k0, k1) tile-pos ranges


def wrap16(idx):
    n = len(idx)
    assert n % 16 == 0
    a = idx.reshape(n // 16, 16).T.astype(np.int16)
    return np.tile(a, (8, 1))


def prepare(x, W, b, a, edge_index):
    N = x.shape[0]
    E = edge_index.shape[1]
    src = edge_index[0].astype(np.int64)
    dst = edge_index[1].astype(np.int64)

    deg = np.bincount(dst, minlength=NP_)
    sdeg = np.bincount(src, minlength=NP_)

    # --- phase 1: table partition per node (src side) -------------------
    order0 = np.argsort(-deg, kind="stable")
    tiles0 = order0.reshape(N_TILES, P)
    node_part = np.empty(NP_, np.int64)
    OVER = np.arange(47, 81)
    OTHER = np.array([p for p in range(P) if not (47 <= p <= 80)])
    so = np.argsort(-sdeg[tiles0], axis=1, kind="stable")
    for t in range(N_TILES):
        m = tiles0[t]
        o = so[t]
        node_part[m[o[:34]]] = OVER
        node_part[m[o[34:]]] = OTHER

    # --- per-edge window assignment (balance L/H per dst node) ----------
    src_p = node_part[src]
    canL = src_p <= 80
    canH = src_p >= 47
    free = canL & canH
    nLh = np.bincount(dst[canL & ~free], minlength=NP_)
    nHh = np.bincount(dst[canH & ~free], minlength=NP_)
    nF = np.bincount(dst[free], minlength=NP_)
    tot = nLh + nHh + nF
    nLb = np.maximum(nLh, np.minimum(nLh + nF, (tot + 1) // 2))
    nHb = tot - nLb
    # free edges of each dst: first (nLb - nLh) go L, rest H
    eorder = np.lexsort((~free, dst))   # per dst: free edges first
    e_sorted = np.arange(E)[eorder]
    d_sorted = dst[eorder]
    first = np.r_[True, d_sorted[1:] != d_sorted[:-1]]
    starts = np.flatnonzero(first)
    rank = np.arange(E) - np.repeat(starts, np.diff(np.r_[starts, E]))
    isfree_s = free[e_sorted]
    quotaL = (nLb - nLh)[d_sorted]
    toL_s = np.where(isfree_s, rank < quotaL, canL[e_sorted] )
    toL = np.empty(E, bool)
    toL[e_sorted] = toL_s
    assert (toL & ~canL).sum() == 0 and ((~toL) & ~canH).sum() == 0

    # --- phase 2: dst tiling + (pos, core) assignment -------------------
    key = np.lexsort((nHb, nLb, -tot))
    tiles = key.reshape(N_TILES, P)             # [tile, dstpos] -> node
    capL_t = nLb[tiles].max(1)
    capH_t = nHb[tiles].max(1)
    trank = np.argsort(-(capL_t + capH_t), kind="stable")
    # sorted position i -> pos i//8, core i%8
    tile_of = trank.reshape(T_CORE, N_CORES)     # [pos, core] -> tile id
    capL = capL_t[tile_of].max(1)
    capH = capH_t[tile_of].max(1)

    node_tile = np.empty(NP_, np.int64)
    node_dpos = np.empty(NP_, np.int64)
    for t in range(N_TILES):
        node_tile[tiles[t]] = t
        node_dpos[tiles[t]] = np.arange(P)

    # --- groups ---------------------------------------------------------
    groups = []
    k0 = 0
    while k0 < T_CORE:
        c = 0
        k1 = k0
        while k1 < T_CORE and (c == 0 or c + capL[k1] + capH[k1] <= GC):
            c += capL[k1] + capH[k1]
            k1 += 1
        groups.append((k0, k1))
        k0 = k1
    cfg_groups = tuple(groups)

    # --- feature reorder + sign fold ------------------------------------
    pos_f = np.flatnonzero(a >= 0)
    neg_f = np.flatnonzero(a < 0)
    fperm = np.concatenate([pos_f, neg_f])
    kpos = len(pos_f)
    atil = np.abs(a)[fperm]
    Wp = W[fperm]                  # [64 out-perm, 64 in]
    bp = b[fperm]
    WT_aug = np.zeros((D + 1, 2 * D), np.float16)
    WT_aug[:D, 0:D] = (Wp.T * atil).astype(np.float16)
    WT_aug[:D, D:2 * D] = Wp.T.astype(np.float16)
    WT_aug[D, 0:D] = (bp * atil).astype(np.float16)
    WT_aug[D, D:2 * D] = bp.astype(np.float16)

    cfg = Cfg(capL=tuple(int(v) for v in capL),
              capH=tuple(int(v) for v in capH),
              kpos=kpos, groups=cfg_groups)

    # --- per-core data ---------------------------------------------------
    # table t-index per (core, node): per core, per partition, nodes with
    # that partition get t = 0..399. Node's table column in xT = t*128+p.
    xpad = np.zeros((NP_, D), np.float32)
    xpad[:N] = x
    x16 = xpad.astype(np.float16)

    colsL = np.asarray(capL)
    colsH = np.asarray(capH)
    CC = int((colsL + colsH).sum())

    # edge sort: by (tile, window(toL first), dstpos, anything)
    in_maps = []
    # node table index t: shared across cores (no per-core constraint)
    t_of = np.empty(NP_, np.int64)
    for p in range(P):
        nodes_p = np.flatnonzero(node_part == p)
        assert len(nodes_p) == N_TILES
        t_of[nodes_p] = np.arange(N_TILES)
    row = node_part * N_TILES + t_of            # table row
    # xT column (t*128+p) -> node
    xcol = np.empty(NP_, np.int64)
    xcol[t_of * P + node_part] = np.arange(NP_)
    xT = np.ascontiguousarray(x16[xcol].T)      # [64, 51200] f16
    xT_aug = np.concatenate([xT, np.ones((1, NP_), np.float16)])

    # per-core edge slot tables
    e_tile = node_tile[dst]
    e_core = np.empty(E, np.int64)
    e_pos = np.empty(E, np.int64)
    # tile id -> (pos, core)
    tpos = np.empty(N_TILES, np.int64)
    tcore = np.empty(N_TILES, np.int64)
    for i in range(N_TILES):
        tcore[trank[i]] = i % N_CORES
        tpos[trank[i]] = i // N_CORES
    e_core = tcore[e_tile]
    e_pos = tpos[e_tile]
    e_dpos = node_dpos[dst]

    # column base offsets per (pos, window) in the group-local layout
    colbaseL = np.zeros(T_CORE, np.int64)
    colbaseH = np.zeros(T_CORE, np.int64)
    gstartL = {}
    gstartH = {}
    off = 0
    for (k0g, k1g) in cfg_groups:
        cL = int(colsL[k0g:k1g].sum())
        cH = int(colsH[k0g:k1g].sum())
        o = 0
        for k in range(k0g, k1g):
            colbaseL[k] = off + o
            o += colsL[k]
        for k in range(k0g, k1g):
            colbaseH[k] = off + o
            o += colsH[k]
        gstartL[k0g] = off
        gstartH[k0g] = off + cL
        off += cL + cH
    assert off == CC

    # per-core rank of edge within (dst node, window)
    for c in range(N_CORES):
        m = e_core == c
        ed = dst[m]
        es = src[m]
        eL = toL[m]
        ep = e_pos[m]
        edp = e_dpos[m]
        okey = np.lexsort((es, ~eL, ed))
        dk = ed[okey]; wk = eL[okey]
        bnd = np.r_[True, (dk[1:] != dk[:-1]) | (wk[1:] != wk[:-1])]
        st = np.flatnonzero(bnd)
        rk = np.arange(len(dk)) - np.repeat(st, np.diff(np.r_[st, len(dk)]))
        rank_e = np.empty(m.sum(), np.int64)
        rank_e[okey] = rk

        col = np.where(eL, colbaseL[ep] + rank_e, colbaseH[ep] + rank_e)
        slot = col * P + edp
        idx_full = np.zeros(CC * P, np.int64)          # default 0 (pad)
        r_e = row[es]
        idx_full[slot] = np.where(eL, r_e, r_e - HBASE)
        maskp = np.zeros(CC * P, np.float16)
        maskp[slot] = 1.0

        idxL_parts = []
        idxH_parts = []
        for (k0g, k1g) in cfg_groups:
            cL = int(colsL[k0g:k1g].sum())
            cH = int(colsH[k0g:k1g].sum())
            s0 = gstartL[k0g] * P
            idxL_parts.append(wrap16(idx_full[s0:s0 + cL * P]))
            s1 = gstartH[k0g] * P
            idxH_parts.append(wrap16(idx_full[s1:s1 + cH * P]))
        idxL = np.concatenate(idxL_parts, axis=1) if idxL_parts else \
            np.zeros((P, 0), np.int16)
        idxH = np.concatenate(idxH_parts, axis=1) if idxH_parts else \
            np.zeros((P, 0), np.int16)

        # local dst-tile x (dst-arranged): columns = (pos k, dpos p)
        own_nodes = tiles[tile_of[:, c]].reshape(-1)   # [50*128]
        xloc = np.ascontiguousarray(x16[own_nodes].T)
        xloc_aug = np.concatenate([xloc, np.ones((1, T_CORE * P), np.float16)])

        in_maps.append({
            "xT": xT_aug, "xTloc": xloc_aug, "WT": WT_aug,
            "idxL": idxL, "idxH": idxH,
            "maskp": np.ascontiguousarray(
                maskp.reshape(CC, P).T),
        })

    meta = {"N": N, "fperm": fperm, "tiles": tiles, "tile_of": tile_of,
            "cfg": cfg}
    return cfg, in_maps, meta


def build(cfg: Cfg, reps=1, stage="full"):
    # stage: ablation level — "gather", "score", "nored", "full"
    S_GATHER = 0
    S_SCORE = 1
    S_TREE = 2
    S_NORED = 3
    S_FULL = 4
    slvl = {"gather": 0, "score": 1, "tree": 2, "nored": 3, "full": 4}[stage]
    nc = bacc.Bacc("TRN2", target_bir_lowering=False, debug=False,
                   num_devices=N_CORES, num_swdge_queues=4)
    capL, capH = cfg.capL, cfg.capH
    groups = cfg.groups
    kpos = cfg.kpos
    CC = sum(capL) + sum(capH)

    xT_d = nc.dram_tensor("xT", [D + 1, NP_], F16, kind="ExternalInput").ap()
    xTl_d = nc.dram_tensor("xTloc", [D + 1, T_CORE * P], F16,
                           kind="ExternalInput").ap()
    WT_d = nc.dram_tensor("WT", [D + 1, 2 * D], F16, kind="ExternalInput").ap()
    idxL_d = nc.dram_tensor("idxL", [P, sum(capL) * 8], I16,
                            kind="ExternalInput").ap()
    idxH_d = nc.dram_tensor("idxH", [P, sum(capH) * 8], I16,
                            kind="ExternalInput").ap()
    maskp_d = nc.dram_tensor("maskp", [P, CC], F16,
                             kind="ExternalInput").ap()
    out_d = nc.dram_tensor("out", [P, T_CORE * D], F16,
                           kind="ExternalOutput").ap()
    wh_t = nc.dram_tensor("wh", [P, N_TILES, 2 * D], F16)
    wh_d = wh_t.ap()
    wh_flat = wh_t.ap().rearrange("p t f -> (p t) f")

    with tile.TileContext(nc) as tc:
        with ExitStack() as ctx:
            cpool = ctx.enter_context(tc.tile_pool(name="const", bufs=1))
            WT_sb = cpool.tile([D + 1, 2 * D], F16)
            nc.sync.dma_start(WT_sb[:], WT_d[:, :])
            whloc = cpool.tile([P, T_CORE, D], F16)
            rpool = ctx.enter_context(tc.tile_pool(name="repstate", bufs=2))
            sumL = sum(capL)
            sumH = sum(capH)
            iL_sb = cpool.tile([P, sumL * 8], I16)
            nc.sync.dma_start(iL_sb[:], idxL_d[:, :])
            iH_sb = cpool.tile([P, sumH * 8], I16)
            nc.sync.dma_start(iH_sb[:], idxH_d[:, :])
            mask_sb = cpool.tile([P, CC], F16)
            nc.sync.dma_start(mask_sb[:], maskp_d[:, :])


            # ---- wh_stage: full table + local scaled tiles -------------
            with ExitStack() as c2:
                xp = c2.enter_context(tc.tile_pool(name="xt", bufs=3))
                pp = c2.enter_context(tc.tile_pool(name="whps", bufs=3,
                                                   space="PSUM"))
                sp = c2.enter_context(tc.tile_pool(name="whsb", bufs=3))
                GT = 8
                for g in range(N_TILES // GT):
                    t0 = g * GT
                    xt = xp.tile([D + 1, GT * P], F16, tag="xt")
                    nc.sync.dma_start(xt[:], xT_d[:, t0 * P:(t0 + GT) * P])
                    ps = pp.tile([P, GT, 2 * D], F32, tag="ps")
                    for j in range(GT):
                        nc.tensor.matmul(ps[:, j, :],
                                         lhsT=xt[:, j * P:(j + 1) * P],
                                         rhs=WT_sb[:], start=True, stop=True)
                    st = sp.tile([P, GT, 2 * D], F16, tag="st")
                    if g % 2 == 0:
                        nc.vector.tensor_copy(st[:], ps[:])
                    else:
                        nc.scalar.activation(st[:], ps[:], AF.Identity)
                    nc.sync.dma_start(wh_d[:, t0:t0 + GT, :], st[:])
                # local pass: 50 tiles dst-arranged, keep scaled half
                for g in range(7):
                    t0 = g * GT
                    nt = min(GT, T_CORE - t0)
                    xt = xp.tile([D + 1, GT * P], F16, tag="xt")
                    nc.sync.dma_start(xt[:, 0:nt * P],
                                      xTl_d[:, t0 * P:(t0 + nt) * P])
                    ps = pp.tile([P, GT, 2 * D], F32, tag="ps")
                    for j in range(nt):
                        nc.tensor.matmul(ps[:, j, :],
                                         lhsT=xt[:, j * P:(j + 1) * P],
                                         rhs=WT_sb[:], start=True, stop=True)
                    if g % 2 == 0:
                        nc.vector.tensor_copy(whloc[:, t0:t0 + nt, :],
                                              ps[:, 0:nt, 0:D])
                    else:
                        nc.scalar.activation(whloc[:, t0:t0 + nt, :],
                                             ps[:, 0:nt, 0:D], AF.Identity)

            gp = ctx.enter_context(tc.tile_pool(name="gath", bufs=2))
            up = ctx.enter_context(tc.tile_pool(name="u", bufs=2))
            vp = ctx.enter_context(tc.tile_pool(name="v", bufs=2))
            ssp = ctx.enter_context(tc.tile_pool(name="score", bufs=2))
            ap_ = ctx.enter_context(tc.tile_pool(name="aggs", bufs=2))

            goff = []
            offL, offH, offC = 0, 0, 0
            for (k0, k1) in groups:
                cL = sum(capL[k0:k1])
                cH = sum(capH[k0:k1])
                goff.append((offL, offH, offC, cL, cH))
                offL += cL
                offH += cH
                offC += cL + cH

            def emit_front(gi, st):
                """loads + gathers + u-add + prelu for group gi."""
                k0, k1 = groups[gi]
                oL, oH, oC, cL, cH = goff[gi]
                cols = cL + cH
                iL = iL_sb[:, oL * 8:(oL + cL) * 8]
                iH = iH_sb[:, oH * 8:(oH + cH) * 8]

                w = gp.tile([P, GC, 2 * D], F16, tag="w")
                q = gi % 4
                pieces = []
                for (base, cn, isL) in ((0, cL, True), (cL, cH, False)):
                    if cn == 0:
                        continue
                    ns = min(GSPLIT, cn)
                    bnd = [cn * i // ns for i in range(ns + 1)]
                    for i in range(ns):
                        if bnd[i + 1] > bnd[i]:
                            pieces.append((base + bnd[i], base + bnd[i + 1],
                                           bnd[i], isL))
                for (c0, c1, i0, isL) in pieces:
                    nn = c1 - c0
                    it = iL if isL else iH
                    src = wh_flat[0:LWIN, :] if isL else wh_flat[HBASE:NP_, :]
                    nc.gpsimd.dma_gather(
                        out_ap=w[:, c0:c1, :], in_ap=src,
                        idxs_ap=it[:, i0 * 8:(i0 + nn) * 8], num_idxs=nn * P,
                        num_idxs_reg=nn * P, elem_size=2 * D,
                        single_packet=SINGLE_PACKET, queue_num=q)
                    q = (q + 1) % 4
                if st < S_SCORE:
                    return (w, None, None)
                # early raw copy so w recycles without waiting the chain
                v = vp.tile([P, GC, D + 1], F16, tag="v")
                nc.vector.tensor_copy(v[:, 0:cols, 1:D + 1],
                                      w[:, 0:cols, D:2 * D])

                u = up.tile([P, GC, D], F16, tag="u")
                o = 0
                for cap in (capL, capH):
                    for k in range(k0, k1):
                        if cap[k]:
                            nc.vector.tensor_add(
                                u[:, o:o + cap[k], :],
                                w[:, o:o + cap[k], 0:D],
                                whloc[:, k:k + 1, :]
                                .to_broadcast((P, cap[k], D)))
                            o += cap[k]
                # p = a_f * LeakyReLU(s_f) for every feature, sign folded:
                #   F+ (a>=0, scaled by |a| in the table): Prelu_0.2(u)
                #   F- : -Prelu_0.2(u) == Prelu_5(-0.2 u)
                nc.scalar.activation(u[:, 0:cols, 0:kpos],
                                     u[:, 0:cols, 0:kpos],
                                     AF.Prelu, alpha=NSLOPE)
                nc.scalar.activation(u[:, 0:cols, kpos:D],
                                     u[:, 0:cols, kpos:D],
                                     AF.Prelu, alpha=1.0 / NSLOPE,
                                     scale=-NSLOPE)
                return (v, u, None)

            cur_agg = [None]

            def emit_back(gi, st, handles):
                """score + v + reductions for group gi."""
                agg = cur_agg[0]
                if st < S_TREE:
                    return
                k0, k1 = groups[gi]
                oL, oH, oC, cL, cH = goff[gi]
                cols = cL + cH
                v, u, _ = handles
                e = ssp.tile([P, GC], F16, tag="e")
                with nc.allow_low_precision(reason="f16 score sum"):
                    nc.vector.tensor_reduce(e[:, 0:cols], u[:, 0:cols, :],
                                            axis=mybir.AxisListType.X,
                                            op=mybir.AluOpType.add)
                ex = ssp.tile([P, GC], F16, tag="ex")
                nc.scalar.activation(ex[:, 0:cols], e[:, 0:cols], AF.Exp)
                # v[:,:,0] = masked exp; v[:,:,1:] *= it (raw copied early)
                nc.vector.tensor_mul(v[:, 0:cols, 0], ex[:, 0:cols],
                                     mask_sb[:, oC:oC + cols])
                if st < S_NORED:
                    return
                nc.vector.tensor_mul(
                    v[:, 0:cols, 1:D + 1], v[:, 0:cols, 1:D + 1],
                    v[:, 0:cols, 0].unsqueeze(2)
                    .to_broadcast((P, cols, D)))
                if st < S_FULL:
                    return
                o = 0
                written = set()
                for cap in (capL, capH):
                    for k in range(k0, k1):
                        c = cap[k]
                        if c == 0:
                            continue
                        vv = v[:, o:o + c, :].transpose((0, 2, 1))
                        if k not in written:
                            written.add(k)
                            nc.vector.tensor_reduce(
                                agg[:, k, :], vv,
                                axis=mybir.AxisListType.X,
                                op=mybir.AluOpType.add)
                        else:
                            t2 = ap_.tile([P, D + 1], F32, tag="t2")
                            nc.vector.tensor_reduce(
                                t2[:], vv, axis=mybir.AxisListType.X,
                                op=mybir.AluOpType.add)
                            nc.vector.tensor_add(agg[:, k, :],
                                                 agg[:, k, :], t2[:])
                        o += c

            ng = len(groups)
            zerok = [k for k in range(T_CORE) if capL[k] + capH[k] == 0]
            for rep in range(reps):
                agg = rpool.tile([P, T_CORE, D + 1], F32, tag="agg")
                cur_agg[0] = agg
                for k in zerok:
                    nc.vector.memset(agg[:, k, :], 0.0)
                handles = {}
                for s in range(ng + 1):
                    if s < ng:
                        handles[s] = emit_front(s, slvl)
                    if s >= 1:
                        emit_back(s - 1, slvl, handles.pop(s - 1))

                # tail: rec = 1 / max(den, eps)
                rec = rpool.tile([P, T_CORE], F32, tag="rec")
                obuf = rpool.tile([P, T_CORE, D], F16, tag="obuf")
                nc.vector.tensor_scalar_max(rec[:], agg[:, :, 0], 1e-9)
                nc.vector.reciprocal(rec[:], rec[:])
                for k in range(T_CORE):
                    nc.scalar.activation(obuf[:, k, :], agg[:, k, 1:D + 1],
                                         AF.Sigmoid, scale=rec[:, k:k + 1])
                nc.sync.dma_start(
                    out_d[:, :], obuf[:].rearrange("p t f -> p (t f)"))

    nc.compile()
    return nc


_CACHE = {}


def kernel(x, W, b, a, edge_index):
    x = np.ascontiguousarray(np.asarray(x, dtype=np.float32))
    W = np.ascontiguousarray(np.asarray(W, dtype=np.float32))
    b = np.ascontiguousarray(np.asarray(b, dtype=np.float32))
    a = np.ascontiguousarray(np.asarray(a, dtype=np.float32))
    edge_index = np.asarray(edge_index)

    cfg, in_maps, meta = prepare(x, W, b, a, edge_index)
    nc = _CACHE.get(cfg)
    if nc is None:
        nc = build(cfg)
        _CACHE[cfg] = nc

    from concourse.bass_utils import run_bass_kernel_spmd
    res = run_bass_kernel_spmd(nc, in_maps, core_ids=list(range(N_CORES)))

    N = meta["N"]
    fperm = meta["fperm"]
    tiles = meta["tiles"]
    tile_of = meta["tile_of"]
    inv_f = np.argsort(fperm)
    y = np.empty((NP_, D), np.float32)
    for c in range(N_CORES):
        o = np.asarray(res.results[c]["out"]).reshape(P, T_CORE, D)
        own = tiles[tile_of[:, c]]              # [50, 128]; o[p,k]=own[k,p]
        y[own.transpose(1, 0).reshape(-1)] = o.reshape(-1, D)
    return y[:N][:, inv_f].astype(np.float32)

